# revision 1
# baseline (speedup 1.0000x reference)
"""Trainium2 Bass kernel for nn_Attention (B=4, N=2048, C=768, H=12).

Sharding: 8 cores = 4 batches x 2 head-groups (6 heads each), Megatron-style
tensor parallel on the heads. Each core computes qkv for its head slice,
attention for 6 heads, and per-head-pair output-projection partials
out3 [3, 2048, 768]. The host sums the 3 pair partials of the 2 cores
covering each batch and adds the bias.

Per-core attention scheme (no transposes anywhere):
  - q,k stored [d, n] (feature-major) straight out of the QKV matmul; heads
    packed in pairs per 128-partition group (head 2p -> partitions 0-63,
    head 2p+1 -> 64-127).
  - S^T tiles [128 j, i] = k_chunk.T @ q  (K=64 matmul). exp() on scalar
    engine reads PSUM, writes SBUF. No max subtraction (logits are O(10);
    softmax is shift-invariant so this only perturbs rounding).
  - v stored [n, d] with an extra ones column; PV matmul lhsT=v[j,0:65],
    rhs=exp(S^T) accumulates [65, 512] where row 64 = sum_j exp = Z.
  - normalize: 1/Z broadcast across partitions via a DRAM-bounce DMA, one
    DVE multiply; odd heads' results are DMA'd up to partitions 64-127
    (engines cannot shift partitions; DMA can).

Matmuls default to float32r (TF32-class, 4x faster than fp32 on the PE;
measured 3.8e-4 scale-relative absmax error vs the fp32 reference).
Set KERNEL_MM_DT=float32 for full fp32 precision (3.4e-6) at ~3x the time.
"""

import os
import sys
from contextlib import ExitStack

if "/opt/trn_rl_repo" not in sys.path:
    sys.path.insert(0, "/opt/trn_rl_repo")

import numpy as np

import concourse.bass as bass
import concourse.mybir as mybir
import concourse.tile as tile
from concourse import bass_utils

F32 = mybir.dt.float32

B, N, C = 4, 2048, 768
NH, D = 12, 64
SCALE = D ** -0.5
HPC = NH // 2          # heads per core
F = HPC * D            # 384 per-core features per projection
QKVF = 3 * F           # 1152
P = 128
CO = C // P            # 6 contraction chunks
FO = F // P            # 3 feature chunks (head pairs)
NO = N // P            # 16 token chunks of 128
NCORES = 8

_MM_DT_NAME = os.environ.get("KERNEL_MM_DT", "float32r")
MM_DT = getattr(mybir.dt, _MM_DT_NAME)


def _d(ap):
    """Cast an fp32 AP to the matmul compute dtype (bitcast, same bytes)."""
    return ap.bitcast(MM_DT) if MM_DT != F32 else ap


def _r(ap):
    """Cast a producer OUT AP feeding a matmul to the compute dtype, so the
    producing engine rounds to fp32r (walrus verifies this chain)."""
    return ap.bitcast(MM_DT) if MM_DT == mybir.dt.float32r else ap


def _split_multiwaits(nc):
    """This container's walrus accepts at most ONE sync-wait per instruction.

    Split any instruction carrying N>1 waits into (N-1) single-wait NOPs on
    the same engine queue placed immediately before it (engine queues are
    FIFO, so the semantics are identical)."""
    ctr = 0
    for f in nc.m.functions:
        for blk in f.blocks:
            insts = blk.instructions
            out = []
            changed = False
            for ins in insts:
                si = ins.sync_info
                if si is not None and len(si.on_wait) > 1:
                    changed = True
                    waits = list(si.on_wait)
                    for ww in waits[:-1]:
                        nop = mybir.InstNoOp(name=f"zzsplitw_{ctr}", ins=[], outs=[])
                        ctr += 1
                        nop.engine = ins.engine
                        nop.sync_info = mybir.SyncInfo(on_wait=[ww], on_update=[])
                        out.append(nop)
                    ins.sync_info = mybir.SyncInfo(
                        on_wait=[waits[-1]], on_update=list(si.on_update)
                    )
                out.append(ins)
            if changed:
                blk.instructions = out
    return nc


def _emit(nc, tc, ctx):
    # x pre-chunked host-side to [co][n4][128, 512] so every slice DMA is
    # one fully-contiguous 256KB read
    xTc = nc.dram_tensor("xTc", [CO, 4, P, 512], F32, kind="ExternalInput").ap()
    # five contiguous weight sections (fully linear DMA reads; a single
    # [C, 1152] tensor would make every section load a 512B-strided gather
    # during the bandwidth-bound lead-in)
    wq_secs = {
        lo: nc.dram_tensor(f"wq{lo}", [C, hi - lo], F32, kind="ExternalInput").ap()
        for lo, hi in ((0, P), (F, F + P), (2 * F, 3 * F), (P, F), (F + P, 2 * F))
    }
    wprojT = nc.dram_tensor("wprojT", [F, C], F32, kind="ExternalInput").ap()
    out3 = nc.dram_tensor("out3", [FO, N, C], F32, kind="ExternalOutput").ap()

    persist = ctx.enter_context(tc.tile_pool(name="persist", bufs=1))

    # q/k in [feature, token] layout, split per (pair, 512-token chunk) so
    # consumers wait only on the producer they actually need (Tile tracks
    # dependencies at whole-tile granularity).
    q_sb = [[persist.tile([P, 512], F32, tag=f"q{fo}_{n4}", name=f"q{fo}_{n4}")
             for n4 in range(4)] for fo in range(FO)]
    k_sb = [[persist.tile([P, 512], F32, tag=f"k{fo}_{n4}", name=f"k{fo}_{n4}")
             for n4 in range(4)] for fo in range(FO)]
    # v in [token, feature] layout per 128-token chunk, +1 ones column.
    v_sb = [persist.tile([P, HPC, D + 1], F32, tag=f"v{no}", name=f"v{no}") for no in range(NO)]
    # attention output per pair, [feature, token] layout; 2 rotating slots
    # (pair 2 reuses pair 0's slot once proj-0 has drained it)
    otp = ctx.enter_context(tc.tile_pool(name="otp", bufs=2))
    ot_sb = [otp.tile([P, N], F32, tag="ot", name=f"ot{pr}") for pr in range(FO)]
    wp_sb = persist.tile([P, FO, C], F32, tag="wp")

    ones_sb = persist.tile([P, HPC], F32, tag="ones")
    nc.vector.memset(ones_sb, 1.0)
    for no in range(NO):
        # DVE copy (not memset) so the output can be declared fp32r
        nc.vector.tensor_copy(out=_r(v_sb[no][:, :, D : D + 1]), in_=ones_sb)
    # dummy exp: pulls the ~2.7us ACT table load into the DMA lead-in window
    expwarm = persist.tile([P, HPC], F32, tag="expwarm")
    nc.scalar.activation(
        out=expwarm,
        in_=ones_sb,
        func=mybir.ActivationFunctionType.Exp,
        scale=1.0,
    )

    with (
        tc.tile_pool(name="wqp", bufs=1) as wqp,
        tc.tile_pool(name="xs", bufs=4) as xs_pool,
        tc.tile_pool(name="ptp", bufs=3) as pt_pool,
        tc.tile_pool(name="rp", bufs=2) as r_pool,
        tc.tile_pool(name="outp", bufs=3) as outp,
        tc.tile_pool(name="rd", bufs=3, space="DRAM") as rd_pool,
        tc.tile_pool(name="ps1", bufs=2, space="PSUM") as ps1,
        tc.tile_pool(name="ps_st", bufs=2, space="PSUM") as ps_st,
        tc.tile_pool(name="ps_o", bufs=2, space="PSUM") as ps_o,
    ):
        # weight tiles per (column-section, contraction chunk) so each qkv
        # matmul depends on exactly one DMA
        wq_tiles = {}

        def load_wq(slices):
            for lo, hi in slices:
                for co in range(CO):
                    t = wqp.tile([P, hi - lo], F32, tag=f"wq_{lo}_{co}",
                                 name=f"wq_{lo}_{co}")
                    wq_tiles[(lo, co)] = t
                    nc.sync.dma_start(
                        out=_r(t),
                        in_=_r(wq_secs[lo][co * P : (co + 1) * P, :]),
                    )

        def wq_slice(foff, co, width=P):
            """AP for weight columns [foff, foff+width) of chunk co."""
            for lo, hi in ((0, P), (F, F + P), (2 * F, 3 * F), (P, F), (F + P, 2 * F)):
                if lo <= foff and foff + width <= hi:
                    return wq_tiles[(lo, co)][:, foff - lo : foff - lo + width]
            raise KeyError(foff)

        def emit_qkv_pass(fo, pre_xt4=None):
            """q/k chunk fo over all tokens.

            Streams xT per 512-token slice (xT is re-read from DRAM once per
            pass; DMA is far from the bottleneck and this keeps SBUF free)."""
            xt4s = []
            for n4 in range(4):
                if n4 == 0 and pre_xt4 is not None:
                    # tile + DMAs already emitted (interleaved with weights);
                    # still run this n4's compute below
                    xt4 = pre_xt4
                    xt4s.append(xt4)
                else:
                    xt4 = xs_pool.tile([P, CO, 512], F32, tag="xt4",
                                       name=f"xt4_{fo}_{n4}")
                    xt4s.append(xt4)
                    # one DMA per contraction chunk so the first matmul can
                    # start after ~256KB instead of the full 1.5MB slice
                    for co in range(CO):
                        nc.sync.dma_start(
                            out=_r(xt4[:, co, :]),
                            in_=_r(xTc[co, n4, :, :]),
                        )
                for dst, foff in ((q_sb[fo][n4], fo * P), (k_sb[fo][n4], F + fo * P)):
                    pq = ps1.tile([P, 512], F32, tag="pqk")
                    for co in range(CO):
                        nc.tensor.matmul(
                            pq,
                            _d(wq_slice(foff, co)),
                            _d(xt4[:, co, :]),
                            start=(co == 0),
                            stop=(co == CO - 1),
                        )
                    nc.vector.tensor_copy(out=_r(dst), in_=pq)
            return xt4s

        def emit_v_chunk(no, xtv):
            """v for one 128-token chunk, reading an [P, CO, 512] x-slice."""
            pv = ps1.tile([P, F], F32, tag="pqk", name=f"pv_{no}")
            for co in range(CO):
                nc.tensor.matmul(
                    pv,
                    _d(xtv[:, co, (no % 4) * P : (no % 4 + 1) * P]),
                    _d(wq_slice(2 * F, co, F)),
                    start=(co == 0),
                    stop=(co == CO - 1),
                )
            nc.vector.tensor_copy(
                out=_r(v_sb[no][:, :, 0:D]),
                in_=pv.rearrange("p (h d) -> p h d", h=HPC),
            )

        def emit_normalize(po, pr, plo, i512):
            # evacuate PSUM -> SBUF at once so the po slot frees for the next
            # i512 block (the normalize chain below has DMA latency in it)
            ov = r_pool.tile([65, 512], F32, tag="ov", name=f"ov_{pr}_{plo}_{i512}")
            # 1/Z lives on partition 64 (engines cannot move data across
            # partitions, so compute in place on lane 64); reading po directly
            # lets the broadcast DMA start before the row evacuation finishes
            nc.vector.reciprocal(out=ov[64:65, :], in_=po[64:65, :])
            nc.vector.tensor_copy(out=ov[0:64, :], in_=po[0:64, :])
            # partition-broadcast 1/Z: SBUF zero-step partition APs are
            # illegal, so bounce through DRAM (DRAM APs broadcast fine)
            rdram = rd_pool.tile([1, 512], F32, tag="rd", name=f"rd_{pr}_{plo}_{i512}")
            nc.sync.dma_start(out=rdram, in_=ov[64:65, :])
            rb = r_pool.tile([64, 512], F32, tag="rb", name=f"rb_{pr}_{plo}_{i512}")
            nc.sync.dma_start(out=rb, in_=rdram.to_broadcast([64, 512]))
            if plo == 0:
                nc.vector.tensor_mul(
                    out=_r(ot_sb[pr][0:64, i512 : i512 + 512]),
                    in0=ov[0:64, :],
                    in1=rb,
                )
            else:
                # odd head: normalize at partitions 0-63, then DMA up to
                # partitions 64-127 of ot
                nt = r_pool.tile([64, 512], F32, tag="nt", name=f"nt_{pr}_{i512}")
                nc.vector.tensor_mul(out=_r(nt), in0=ov[0:64, :], in1=rb)
                nc.sync.dma_start(
                    out=_r(ot_sb[pr][64:128, i512 : i512 + 512]), in_=_r(nt)
                )

        # interleave: qkv pass for a head pair, then that pair's attention.
        # Both heads of a pair share one [128, 1024] S^T tile (head A cols
        # 0-511, head B cols 512-1023): their K=64 matmuls sit at PE row
        # groups 0-1 / 2-3 and run concurrently, and one exp() covers both.
        def emit_attention(pr, interleave_proj=False, xt4s=None):
            hA, hB = 2 * pr, 2 * pr + 1
            for i4 in range(4):
                i0 = i4 * 512
                po_A = ps_o.tile([65, 512], F32, tag="po", name=f"poA_{pr}_{i4}")
                po_B = ps_o.tile([65, 512], F32, tag="po", name=f"poB_{pr}_{i4}")
                for j in range(NO):
                    kt = k_sb[pr][j // 4]
                    jo = (j % 4) * P
                    qt = q_sb[pr][i4]
                    stm = ps_st.tile([P, 1024], F32, tag="st", name=f"st_{j}")
                    nc.tensor.matmul(
                        stm[:, 0:512],
                        _d(kt[0:64, jo : jo + P]),
                        _d(qt[0:64, :]),
                        start=True,
                        stop=True,
                    )
                    nc.tensor.matmul(
                        stm[:, 512:1024],
                        _d(kt[64:128, jo : jo + P]),
                        _d(qt[64:128, :]),
                        start=True,
                        stop=True,
                    )
                    ptile = pt_pool.tile([P, 1024], F32, tag="pt", name=f"pt_{j}")
                    nc.scalar.activation(
                        out=_r(ptile),
                        in_=stm,
                        func=mybir.ActivationFunctionType.Exp,
                        scale=SCALE,
                    )
                    if xt4s is not None and i4 == 0:
                        # produce v[j] just before its first consumer, reusing
                        # the x slices already in SBUF from the q/k pass; these
                        # matmuls fill PE gaps while the scalar engine exps
                        emit_v_chunk(j, xt4s[j // 4])
                    nc.tensor.matmul(
                        po_A,
                        _d(v_sb[j][:, hA, :]),
                        _d(ptile[:, 0:512]),
                        start=(j == 0),
                        stop=(j == NO - 1),
                    )
                    nc.tensor.matmul(
                        po_B,
                        _d(v_sb[j][:, hB, :]),
                        _d(ptile[:, 512:1024]),
                        start=(j == 0),
                        stop=(j == NO - 1),
                    )
                emit_normalize(po_A, pr, 0, i0)
                emit_normalize(po_B, pr, 64, i0)
                if interleave_proj:
                    emit_proj(pr, no_range=range(4 * i4, 4 * i4 + 4))

        def emit_proj(pr, no_range=None):
            # per-pair projection partial: out3[pr] = ot_pair.T @ wp[pr]
            # (the host sums the three pair partials; this removes the
            # cross-pair barrier and overlaps proj with the next pair)
            for no in (no_range if no_range is not None else range(NO)):
                o_sb = outp.tile([P, C], F32, tag="o", name=f"o_{pr}_{no}")
                for ob, width in ((0, 512), (1, 256)):
                    pp = ps1.tile([P, 512], F32, tag="pqk", name=f"pp_{pr}_{no}_{ob}")
                    nc.tensor.matmul(
                        pp[:, 0:width],
                        _d(ot_sb[pr][:, no * P : (no + 1) * P]),
                        _d(wp_sb[:, pr, ob * 512 : ob * 512 + width]),
                        start=True,
                        stop=True,
                    )
                    nc.vector.tensor_copy(
                        out=o_sb[:, ob * 512 : ob * 512 + width], in_=pp[:, 0:width]
                    )
                nc.sync.dma_start(
                    out=out3[pr, no * P : (no + 1) * P, :], in_=o_sb
                )

        # emission order = scheduling priority. Minimal weights first so
        # compute starts ~10us in; qkv pass pr runs in PE slack during
        # attention pr-1; proj pr-1 runs during attention pr; the last
        # pair's proj interleaves into its own attention blocks.
        # interleave q0/k0 weight DMAs with the first x slice per chunk so
        # the first matmul's operands co-arrive in the DMA queue
        xt4_00 = xs_pool.tile([P, CO, 512], F32, tag="xt4", name="xt4_00")
        for co in range(CO):
            for lo, hi in ((0, P), (F, F + P)):
                t = wqp.tile([P, hi - lo], F32, tag=f"wq_{lo}_{co}",
                             name=f"wq_{lo}_{co}")
                wq_tiles[(lo, co)] = t
                nc.sync.dma_start(
                    out=_r(t), in_=_r(wq_secs[lo][co * P : (co + 1) * P, :])
                )
            nc.sync.dma_start(
                out=_r(xt4_00[:, co, :]), in_=_r(xTc[co, 0, :, :])
            )
        xt4s0 = emit_qkv_pass(0, pre_xt4=xt4_00)
        load_wq([(2 * F, 3 * F)])                      # v (needed ~12us in)
        emit_attention(0, xt4s=xt4s0)
        load_wq([(P, F), (F + P, 2 * F)])              # q1/q2, k1/k2
        for fo in range(FO):
            nc.sync.dma_start(
                out=_r(wp_sb[:, fo, :]),
                in_=_r(wprojT[fo * P : (fo + 1) * P, :]),
            )
        for pr in range(1, FO):
            emit_qkv_pass(pr)
            emit_proj(pr - 1)
            emit_attention(pr, interleave_proj=(pr == FO - 1))


_NC_CACHE = {}


def build_bass():
    key = _MM_DT_NAME
    if key in _NC_CACHE:
        return _NC_CACHE[key]
    nc = bass.Bass("TRN2")
    with tile.TileContext(nc) as tc:
        with ExitStack() as ctx:
            _emit(nc, tc, ctx)
    _split_multiwaits(nc)
    _NC_CACHE[key] = nc
    return nc


def make_in_maps(x, w_qkv, w_proj):
    x = np.asarray(x, dtype=np.float32)
    w_qkv = np.asarray(w_qkv, dtype=np.float32)
    w_proj = np.asarray(w_proj, dtype=np.float32)
    wq, wk, wv = w_qkv[0:C], w_qkv[C : 2 * C], w_qkv[2 * C : 3 * C]
    in_maps = []
    for c in range(NCORES):
        b, g = divmod(c, 2)
        sl = slice(g * F, (g + 1) * F)
        wslice = np.concatenate([wq[sl], wk[sl], wv[sl]], axis=0)  # [1152, 768]
        wT = np.ascontiguousarray(wslice.T)  # [768, 1152]
        xT = x[b].T  # [768, 2048]
        xTc = np.ascontiguousarray(
            xT.reshape(CO, P, 4, 512).transpose(0, 2, 1, 3)
        )  # [co, n4, 128, 512]
        m = {
            "xTc": xTc,
            "wprojT": np.ascontiguousarray(w_proj[:, sl].T),
        }
        for lo, hi in ((0, 128), (384, 512), (768, 1152), (128, 384), (512, 768)):
            m[f"wq{lo}"] = np.ascontiguousarray(wT[:, lo:hi])
        in_maps.append(m)
    return in_maps


def gather_output(parts, b_proj):
    """parts: 8 arrays [FO, N, C] (pair partials per core)."""
    outv = np.empty((B, N, C), np.float32)
    for b in range(B):
        outv[b] = parts[2 * b].sum(axis=0) + parts[2 * b + 1].sum(axis=0)
    outv += np.asarray(b_proj, dtype=np.float32)[None, None, :]
    return outv


def kernel(x, w_qkv, w_proj, b_proj, _run_kwargs=None):
    nc = build_bass()
    in_maps = make_in_maps(x, w_qkv, w_proj)
    res = bass_utils.run_bass_kernel_spmd(
        nc, in_maps, core_ids=list(range(NCORES)), **(_run_kwargs or {})
    )
    parts = [r["out3"] for r in res.results]
    outv = gather_output(parts, b_proj)
    if _run_kwargs is not None:
        kernel.last_results = res
    return outv



# revision 45
# speedup vs baseline: 1.1306x; 1.1306x over previous
"""Trainium2 Bass kernel for nn_Attention (B=4, N=2048, C=768, H=12).

Sharding: 8 cores = 4 batches x 2 head-groups (6 heads each), Megatron-style
tensor parallel on the heads. Each core computes qkv for its head slice,
attention for 6 heads, and per-head-pair output-projection partials
out3 [3, 2048, 768] (bf16). The host sums the 3 pair partials of the 2 cores
covering each batch and adds the bias.

Design (optimized against the TimelineSim cost model, where a matmul costs
out_free_size cycles regardless of K, and the ACT engine costs free_size
cycles at 1.2GHz regardless of dtype):
  - ACT is the hard floor: 6 heads x 2048^2 softmax exps = 192 instructions
    of [128, 1024] ~= 199.5us busy. The whole schedule exists to keep the
    exp stream dense.
  - Everything bf16 on the PE (1 cyc/row, same as fp32r, half the DMA/SBUF).
  - x loaded ONCE (bf16, 4 strided DMAs) and resident; qkv passes re-read
    SBUF, not DRAM.
  - S^T tiles [128 j, 1024] = (1 j-chunk x 512 i x 2 heads); exp reads PSUM
    fp32, writes SBUF bf16.
  - PV is TOKEN-major: out[i, d] = P^T-chunk.T @ [V|1]: 65-row matmuls
    (vs 512-row feature-major) -> PV drops from 196k to 100k PE cycles.
    Z rides along as column 64 via a ones-column in v_sb.
  - normalize per token: reciprocal [128,4] + tensor_scalar_mul with
    per-partition 1/Z (no partition broadcast, no DRAM bounce), then a PE
    transpose (bf16, 128 cyc) flips [t, f] -> [f, t] for the projection.
  - The flat (pair, i4, j) S/exp stream is emitted directly; ALL other PE
    work (qkv passes, v production, PV, normalize, projections) flows
    through a budget-paced FIFO work queue drained between exps, so the
    in-order PE always runs S^T(idx+1) before PV(idx) and never starves ACT.
"""

import os
import sys
from contextlib import ExitStack

if "/opt/trn_rl_repo" not in sys.path:
    sys.path.insert(0, "/opt/trn_rl_repo")


def _env(name, default):
    return os.environ.get(name, default)

import numpy as np

import concourse.bass as bass
import concourse.mybir as mybir
import concourse.tile as tile
from concourse import bass_utils
from concourse.masks import make_identity

F32 = mybir.dt.float32
BF16 = mybir.dt.bfloat16

B, N, C = 4, 2048, 768
NH, D = 12, 64
SCALE = D ** -0.5
HPC = NH // 2          # heads per core
F = HPC * D            # 384 per-core features per projection
P = 128
CO = C // P            # 6 contraction chunks
FO = F // P            # 3 feature chunks (head pairs)
NO = N // P            # 16 token chunks of 128
NCORES = 8

# weight column sections (contiguous DMA reads): q0, k0, v, q12, k12
WSECS = ((0, P), (F, F + P), (2 * F, 3 * F), (P, F), (F + P, 2 * F))


def _split_multiwaits(nc):
    """This container's walrus accepts at most ONE sync-wait per instruction.

    Split any instruction carrying N>1 waits into (N-1) single-wait NOPs on
    the same engine queue placed immediately before it (engine queues are
    FIFO, so the semantics are identical)."""
    ctr = 0
    for f in nc.m.functions:
        for blk in f.blocks:
            insts = blk.instructions
            out = []
            changed = False
            for ins in insts:
                si = ins.sync_info
                if si is not None and len(si.on_wait) > 1:
                    changed = True
                    waits = list(si.on_wait)
                    for ww in waits[:-1]:
                        nop = mybir.InstNoOp(name=f"zzsplitw_{ctr}", ins=[], outs=[])
                        ctr += 1
                        nop.engine = ins.engine
                        nop.sync_info = mybir.SyncInfo(on_wait=[ww], on_update=[])
                        out.append(nop)
                    ins.sync_info = mybir.SyncInfo(
                        on_wait=[waits[-1]], on_update=list(si.on_update)
                    )
                out.append(ins)
            if changed:
                blk.instructions = out
    return nc


def _emit(nc, tc, ctx):
    # host pre-arranges everything partition-major so every input DMA is one
    # fully-contiguous >=1KB descriptor per partition (descriptors under 512B
    # pay a 2x latency multiplier in the DMA engines)
    xTb = nc.dram_tensor("xTb", [4, P, CO * 512], BF16, kind="ExternalInput").ap()
    wq_secs = {
        lo: nc.dram_tensor(f"wq{lo}", [P, CO * (hi - lo)], BF16,
                           kind="ExternalInput").ap()
        for lo, hi in WSECS
    }
    wprojT = nc.dram_tensor("wprojT", [P, FO * C], BF16, kind="ExternalInput").ap()
    out3 = nc.dram_tensor("out3", [FO, N, C], BF16, kind="ExternalOutput").ap()

    persist = ctx.enter_context(tc.tile_pool(name="persist", bufs=1))

    # x resident, one tile per 512-token slice: [128, co, 512]
    xt = [persist.tile([P, CO, 512], BF16, tag=f"x{n4}", name=f"x{n4}")
          for n4 in range(4)]
    # q/k in [feature, token] layout (heads packed in pairs per partition
    # group: head 2p -> partitions 0-63, head 2p+1 -> 64-127)
    q_sb = [[persist.tile([P, 512], BF16, tag=f"q{fo}_{n4}", name=f"q{fo}_{n4}")
             for n4 in range(4)] for fo in range(FO)]
    k_sb = [[persist.tile([P, 512], BF16, tag=f"k{fo}_{n4}", name=f"k{fo}_{n4}")
             for n4 in range(4)] for fo in range(FO)]
    # v in [token, head, feature+1] layout; column 64 is ones so the PV
    # matmul accumulates Z in out[:, 64]
    v_sb = [persist.tile([P, HPC, D + 1], BF16, tag=f"v{no}", name=f"v{no}")
            for no in range(NO)]
    # weights resident: one tile per section, [128, co, width]
    wq_sb = {lo: persist.tile([P, CO, hi - lo], BF16, tag=f"wq{lo}",
                              name=f"wq{lo}") for lo, hi in WSECS}
    wp_sb = persist.tile([P, FO, C], BF16, tag="wp")
    ident = persist.tile([P, P], BF16, tag="ident")
    # attention output per pair, [feature, token]; 2 rotating slots
    otp = ctx.enter_context(tc.tile_pool(name="otp", bufs=3))
    ot_sb = [otp.tile([P, N], BF16, tag="ot", name=f"ot{pr}") for pr in range(FO)]

    make_identity(nc, ident)
    for no in range(NO):
        nc.vector.memset(v_sb[no][:, :, D : D + 1], 1.0)
    # dummy exp: pulls the ACT table load into the DMA lead-in window
    expwarm = persist.tile([P, HPC], F32, tag="expwarm")
    nc.scalar.activation(
        out=expwarm,
        in_=ident[:, 0:HPC],
        func=mybir.ActivationFunctionType.Exp,
        scale=1.0,
    )

    with (
        tc.tile_pool(name="ptp", bufs=24) as pt_pool,
        tc.tile_pool(name="rzp", bufs=2) as rz_pool,
        tc.tile_pool(name="ntp", bufs=2) as nt_pool,
        tc.tile_pool(name="outp", bufs=3) as outp,
        tc.tile_pool(name="ps_st", bufs=2, space="PSUM") as ps_st,
        tc.tile_pool(name="ps_pv", bufs=1, space="PSUM") as ps_pv,
        tc.tile_pool(name="ps_mix", bufs=2, space="PSUM") as ps_mix,
    ):
        def wq_slice(foff, co, width=P):
            """AP for weight columns [foff, foff+width) of chunk co."""
            for lo, hi in WSECS:
                if lo <= foff and foff + width <= hi:
                    return wq_sb[lo][:, co, foff - lo : foff - lo + width]
            raise KeyError(foff)

        # ---- work-item thunks (cost estimates are PE-ns) ---------------
        def qkv_group_thunks(fo, n4, which):
            """One q-or-k feature chunk for one 512-token slice, split into
            3 thunks of 2 matmuls (~430ns PE each)."""
            foff = fo * P if which == "q" else F + fo * P
            dst = (q_sb if which == "q" else k_sb)[fo][n4]
            state = {}

            def mk(c0):
                def th():
                    if c0 == 0:
                        state["pq"] = ps_mix.tile([P, 512], F32, tag="pqk",
                                                  name=f"pq_{which}{fo}_{n4}")
                    pq = state["pq"]
                    for co in (c0, c0 + 1):
                        nc.tensor.matmul(
                            pq,
                            wq_slice(foff, co),
                            xt[n4][:, co, :],
                            start=(co == 0),
                            stop=(co == CO - 1),
                        )
                    if c0 == CO - 2:
                        nc.vector.tensor_copy(out=dst, in_=pq)
                return th

            return [(427, mk(0)), (427, mk(2)), (427, mk(4))]

        def v_chunk_thunks(no):
            """v for one 128-token chunk: 2 thunks of 3 accum matmuls."""
            state = {}

            def mk(c0):
                def th():
                    if c0 == 0:
                        state["pv"] = ps_mix.tile([P, F], F32, tag="pqk",
                                                  name=f"pv_{no}")
                    pv = state["pv"]
                    for co in (c0, c0 + 1, c0 + 2):
                        nc.tensor.matmul(
                            pv,
                            xt[no // 4][:, co, (no % 4) * P : (no % 4 + 1) * P],
                            wq_slice(2 * F, co, F),
                            start=(co == 0),
                            stop=(co == CO - 1),
                        )
                    if c0 == 3:
                        nc.vector.tensor_copy(
                            out=v_sb[no][:, :, 0:D],
                            in_=pv.rearrange("p (h d) -> p h d", h=HPC),
                        )
                        v_done.add(no)
                return th

            return [(480, mk(0)), (480, mk(3))]

        def proj_chunk_thunks(pr, no):
            """Projection partial for one 128-token chunk of pair pr."""
            state = {}

            def mk(ob, width):
                def th():
                    if ob == 0:
                        state["o"] = outp.tile([P, C], BF16, tag="o",
                                               name=f"o_{pr}_{no}")
                    o_sb = state["o"]
                    pp = ps_mix.tile([P, 512], F32, tag="pqk",
                                     name=f"pp_{pr}_{no}_{ob}")
                    nc.tensor.matmul(
                        pp[:, 0:width],
                        ot_sb[pr][:, no * P : (no + 1) * P],
                        wp_sb[:, pr, ob * 512 : ob * 512 + width],
                        start=True,
                        stop=True,
                    )
                    nc.vector.tensor_copy(
                        out=o_sb[:, ob * 512 : ob * 512 + width],
                        in_=pp[:, 0:width],
                    )
                    if ob == 1:
                        nc.sync.dma_start(
                            out=out3[pr, no * P : (no + 1) * P, :], in_=o_sb
                        )
                return th

            return [(213, mk(0, 512)), (107, mk(1, 256))]

        pt_map = {}
        v_done = set()

        # PSUM accumulation groups are exclusive per 2KB zero region (one
        # bank): only ONE group may be open in a bank at a time, and start=
        # True zeroes the whole region. So PV runs as FOUR sequential isub
        # passes per i4 block, each pass holding exactly one open group in
        # the pvA bank and one in the pvB bank; the per-isub normalize
        # (which reads the banks) runs between passes (WAR via the tile
        # pool rotation). The block's 16 exp tiles stay live in pt_pool
        # until its last pass.
        def pv_pass_thunks(pr, i4, isub, tileA=None, tileB=None):
            hA, hB = 2 * pr, 2 * pr + 1
            st = {}

            def mk(t):
                def th():
                    if t == 0:
                        st["A"] = tileA if tileA is not None else ps_pv.tile(
                            [P, D + 1], F32, tag="pvA",
                            name=f"pvA_{pr}_{i4}_{isub}")
                        st["B"] = tileB if tileB is not None else ps_pv.tile(
                            [P, D + 1], F32, tag="pvB",
                            name=f"pvB_{pr}_{i4}_{isub}")
                    pvA, pvB = st["A"], st["B"]
                    for j in range(4 * t, 4 * t + 4):
                        assert j in v_done, f"v({j}) not emitted before PV"
                        ptile = pt_map[(pr, i4, j)]
                        nc.tensor.matmul(
                            pvA,
                            ptile[:, isub * P : (isub + 1) * P],
                            v_sb[j][:, hA, :],
                            start=(j == 0),
                            stop=(j == NO - 1),
                        )
                        nc.tensor.matmul(
                            pvB,
                            ptile[:, 512 + isub * P : 512 + (isub + 1) * P],
                            v_sb[j][:, hB, :],
                            start=(j == 0),
                            stop=(j == NO - 1),
                        )
                    if t == 3:
                        st_norm = norm_one(pr, i4, isub, pvA, pvB)
                        st["norm"] = st_norm
                return th

            return [(220, mk(t)) for t in range(4)]

        def norm_one(pr, i4, isub, pvA, pvB):
            """normalize + transpose one 128-token chunk (reads then frees
            the pv banks). Emitted inline at the end of the pass's last
            thunk so the WAR chain to the next pass is as short as
            possible."""
            i0 = i4 * 512
            rzA = rz_pool.tile([P, 1], F32, tag="rzA",
                               name=f"rzA_{pr}_{i4}_{isub}")
            rzB = rz_pool.tile([P, 1], F32, tag="rzB",
                               name=f"rzB_{pr}_{i4}_{isub}")
            nc.vector.reciprocal(out=rzA, in_=pvA[:, D : D + 1])
            nc.vector.reciprocal(out=rzB, in_=pvB[:, D : D + 1])
            nt = nt_pool.tile([P, P], BF16, tag="nt",
                              name=f"nt_{pr}_{i4}_{isub}")
            nc.vector.tensor_scalar_mul(nt[:, 0:D], pvA[:, 0:D], rzA)
            nc.vector.tensor_scalar_mul(nt[:, D:P], pvB[:, 0:D], rzB)
            tp = ps_mix.tile([P, P], BF16, tag="pqk",
                             name=f"tp_{pr}_{i4}_{isub}")
            nc.tensor.transpose(tp, nt, ident)
            nc.vector.tensor_copy(
                out=ot_sb[pr][:, i0 + isub * P : i0 + (isub + 1) * P],
                in_=tp,
            )

        # ---- DMA lead-in (few, large, contiguous DMAs; the three the first
        # matmuls need go out in parallel on separate engine queues) -------
        def dma_xt(n4, eng=None):
            (eng or nc.sync).dma_start(
                out=xt[n4].rearrange("p co c -> p (co c)"), in_=xTb[n4]
            )

        def dma_wq(lo, eng=None):
            (eng or nc.sync).dma_start(
                out=wq_sb[lo].rearrange("p co c -> p (co c)"), in_=wq_secs[lo]
            )

        dma_wq(0, eng=nc.scalar)       # q0 weights: ACT queue (idle now)
        dma_wq(F, eng=nc.gpsimd)       # k0 weights: gpsimd SWDGE queue
        for cp in range(3):            # x(n4=0) in co-pair pieces: the first
            nc.sync.dma_start(         # q00 matmuls start after ~1 piece
                out=xt[0][:, 2 * cp : 2 * cp + 2, :].rearrange(
                    "p co c -> p (co c)"),
                in_=xTb[0][:, cp * 1024 : (cp + 1) * 1024],
            )
        dma_wq(2 * F)                  # v weights
        dma_xt(1)
        dma_xt(2)
        dma_xt(3)
        dma_wq(P)                      # q1/q2
        dma_wq(F + P)                  # k1/k2
        nc.sync.dma_start(
            out=wp_sb.rearrange("p fo c -> p (fo c)"), in_=wprojT
        )

        # PE p-state warmup: dummy transposes keep the tensor engine busy
        # from t~0 so the clock is ramped when the real matmuls arrive
        for w in range(44):
            wtp = ps_st.tile([P, P], BF16, tag="st", name=f"warm{w}")
            nc.tensor.transpose(wtp, ident, ident)

        # minimal pre-attention compute: q0 for the first 512 tokens, then
        # k0(n4=0) in four token-quarter groups, each in its OWN psum tile
        # (one open accumulation group per 2KB zero region) so S^T(j=0)
        # only waits for the first quarter
        for _, th in qkv_group_thunks(0, 0, "q"):
            th()
        for quarter in range(4):
            sl = slice(quarter * P, (quarter + 1) * P)
            k00p = ps_mix.tile([P, P], F32, tag="pqk", name=f"k00p{quarter}")
            for co in range(CO):
                nc.tensor.matmul(
                    k00p,
                    wq_slice(F, co),
                    xt[0][:, co, sl],
                    start=(co == 0),
                    stop=(co == CO - 1),
                )
            nc.vector.tensor_copy(out=k_sb[0][0][:, sl], in_=k00p)

        # ---- EDF-ordered, budget-paced work pool -----------------------
        # Every non-S^T/exp instruction flows through one pool. Items carry
        # an execution DEADLINE in exp-index units (when their absence would
        # stall the exp stream: S^T operand production, pt-slot recycling
        # via PV, psum WAR via normalize) and an optional nbi (PV(j) may not
        # be emitted before exp(j)). drain() emits earliest-deadline-first,
        # rate-limited so the in-order PE always reaches the next S^T in
        # time; items with imminent deadlines are emitted regardless.
        import heapq

        pool = []          # heap of (deadline, seq, cost, thunk)
        pending = {}       # nbi -> [(deadline, seq, cost, thunk)]
        seq_ctr = [0]

        def put(pairs, deadline, nbi=None):
            for cost, th in pairs:
                item = (deadline, seq_ctr[0], cost, th)
                seq_ctr[0] += 1
                if nbi is not None:
                    pending.setdefault(nbi, []).append(item)
                else:
                    heapq.heappush(pool, item)

        # virtual PE clock: vpe tracks emitted PE-ns (S^T included). The PE
        # is assumed never more than LAG behind the ACT line (idx*1038), so
        # vpe is floored to that before draining; drains stop when the next
        # S^T would land later than RESERVE before its exp slot.
        ACT_NS = 1038.0
        LAG = float(_env("KERNEL_LAG", 1400))
        RESERVE = float(_env("KERNEL_RESERVE", 380))
        vpe = [0.0]

        def drain(idx):
            for k in [k for k in pending if k <= idx]:
                for item in pending.pop(k):
                    heapq.heappush(pool, item)
            vpe[0] = max(vpe[0], idx * ACT_NS - LAG)
            ceil = (idx + 1) * ACT_NS - RESERVE
            while pool:
                deadline, _, cost, th = pool[0]
                if deadline > idx + 1 and vpe[0] + cost > ceil:
                    break
                heapq.heappop(pool)
                th()
                vpe[0] += cost

        def g_idx(pr, i4, j):
            return 64 * pr + 16 * i4 + j

        # ---- the flat S/exp stream with paced drains -------------------
        for pr in range(FO):
            for i4 in range(4):
                for j in range(NO):
                    gi = g_idx(pr, i4, j)
                    # work enqueues; deadlines are the exp idx by which the
                    # item must have EXECUTED to keep the exp stream dense
                    if pr == 0 and i4 == 0:
                        if j == 0:
                            put(qkv_group_thunks(0, 1, "k"), deadline=3)
                            put(qkv_group_thunks(0, 2, "k"), deadline=7)
                            put(qkv_group_thunks(0, 3, "k"), deadline=11)
                            put(qkv_group_thunks(0, 1, "q"), deadline=14)
                            put(qkv_group_thunks(0, 2, "q"), deadline=30)
                            put(qkv_group_thunks(0, 3, "q"), deadline=46)
                        # deadline strictly before any PV pass thunk (the
                        # passes read v_sb; emission order IS dependency
                        # order, so v must pop first)
                        put(v_chunk_thunks(j), deadline=j + 4)
                    elif pr == 0 and i4 == 1 and j == 0:
                        for n4 in range(4):
                            put(qkv_group_thunks(1, n4, "k"),
                                deadline=62 + 4 * n4)
                        for n4 in range(4):
                            put(qkv_group_thunks(1, n4, "q"),
                                deadline=62 + 16 * n4)
                    elif pr == 1 and i4 == 1 and j == 0:
                        for n4 in range(4):
                            put(qkv_group_thunks(2, n4, "k"),
                                deadline=126 + 4 * n4)
                        for n4 in range(4):
                            put(qkv_group_thunks(2, n4, "q"),
                                deadline=126 + 16 * n4)
                    elif pr == 1 and i4 == 0 and j == 0:
                        for no in range(12):
                            put(proj_chunk_thunks(0, no), deadline=900 + no)
                    elif pr == 1 and i4 == 1 and j == 8:
                        # chunks 12-15 need ot[0] i4=3 (normalized early in
                        # pair 1) — enqueued later so EDF can't outrun it
                        for no in range(12, NO):
                            put(proj_chunk_thunks(0, no), deadline=900 + no)
                    elif pr == 2 and i4 == 0 and j == 0:
                        for no in range(12):
                            put(proj_chunk_thunks(1, no), deadline=920 + no)
                    elif pr == 2 and i4 == 1 and j == 8:
                        for no in range(12, NO):
                            put(proj_chunk_thunks(1, no), deadline=920 + no)
                    # the S^T + exp stream itself (emitted directly)
                    kt = k_sb[pr][j // 4]
                    jo = (j % 4) * P
                    qt = q_sb[pr][i4]
                    stm = ps_st.tile([P, 1024], F32, tag="st",
                                     name=f"st_{pr}_{i4}_{j}")
                    nc.tensor.matmul(
                        stm[:, 0:512], kt[0:64, jo : jo + P], qt[0:64, :],
                        start=True, stop=True,
                    )
                    nc.tensor.matmul(
                        stm[:, 512:1024], kt[64:128, jo : jo + P], qt[64:128, :],
                        start=True, stop=True,
                    )
                    ptile = pt_pool.tile([P, 1024], BF16, tag="pt",
                                         name=f"pt_{pr}_{i4}_{j}")
                    nc.scalar.activation(
                        out=ptile,
                        in_=stm,
                        func=mybir.ActivationFunctionType.Exp,
                        scale=SCALE,
                    )
                    pt_map[(pr, i4, j)] = ptile
                    vpe[0] += 427.0  # the two S^T matmuls above
                    drain(gi)
                # PV passes: strictly increasing deadlines keep the
                # pass/normalize WAR sequence ordered through the EDF heap
                g15 = g_idx(pr, i4, NO - 1)
                for isub in range(4):
                    ths = pv_pass_thunks(pr, i4, isub)
                    for t, (cost, th) in enumerate(ths):
                        put([(450 if t == 3 else cost, th)],
                            deadline=g15 + 2 + 4 * isub + t,
                            nbi=g_idx(pr, i4, 4 * t + 3) + 1)
                if pr == 2:
                    for no in range(4 * i4, 4 * i4 + 4):
                        put(proj_chunk_thunks(2, no), deadline=g15 + 20)
        # tail: everything left (final PVs, normalize, proj-2 chunks)
        for k in sorted(pending):
            for item in pending.pop(k):
                heapq.heappush(pool, item)
        while pool:
            heapq.heappop(pool)[3]()


_NC_CACHE = {}


def build_bass():
    key = "v3"
    if key in _NC_CACHE:
        return _NC_CACHE[key]
    nc = bass.Bass("TRN2")
    with tile.TileContext(nc) as tc:
        with ExitStack() as ctx:
            _emit(nc, tc, ctx)
    _split_multiwaits(nc)
    _NC_CACHE[key] = nc
    return nc


def make_in_maps(x, w_qkv, w_proj):
    import ml_dtypes

    bf16 = np.dtype(ml_dtypes.bfloat16)
    x = np.asarray(x, dtype=np.float32)
    w_qkv = np.asarray(w_qkv, dtype=np.float32)
    w_proj = np.asarray(w_proj, dtype=np.float32)
    wq, wk, wv = w_qkv[0:C], w_qkv[C : 2 * C], w_qkv[2 * C : 3 * C]
    in_maps = []
    for c in range(NCORES):
        b, g = divmod(c, 2)
        sl = slice(g * F, (g + 1) * F)
        wslice = np.concatenate([wq[sl], wk[sl], wv[sl]], axis=0)  # [1152, 768]
        wT = np.ascontiguousarray(wslice.T)  # [768, 1152]
        xT = x[b].T  # [768, 2048]
        # [n4, p, co*512]: per-partition rows fully contiguous
        xTb = np.ascontiguousarray(
            xT.reshape(CO, P, 4, 512).transpose(2, 1, 0, 3).reshape(4, P, CO * 512)
        ).astype(bf16)
        wpT = w_proj[:, sl].T  # [384, 768]
        m = {
            "xTb": xTb,
            "wprojT": np.ascontiguousarray(
                wpT.reshape(FO, P, C).transpose(1, 0, 2).reshape(P, FO * C)
            ).astype(bf16),
        }
        for lo, hi in WSECS:
            w = hi - lo
            m[f"wq{lo}"] = np.ascontiguousarray(
                wT[:, lo:hi].reshape(CO, P, w).transpose(1, 0, 2).reshape(P, CO * w)
            ).astype(bf16)
        in_maps.append(m)
    return in_maps


def gather_output(parts, b_proj):
    """parts: 8 arrays [FO, N, C] bf16 (pair partials per core)."""
    outv = np.empty((B, N, C), np.float32)
    for b in range(B):
        acc = parts[2 * b].astype(np.float32).sum(axis=0)
        acc += parts[2 * b + 1].astype(np.float32).sum(axis=0)
        outv[b] = acc
    outv += np.asarray(b_proj, dtype=np.float32)[None, None, :]
    return outv


def kernel(x, w_qkv, w_proj, b_proj, _run_kwargs=None):
    nc = build_bass()
    in_maps = make_in_maps(x, w_qkv, w_proj)
    res = bass_utils.run_bass_kernel_spmd(
        nc, in_maps, core_ids=list(range(NCORES)), **(_run_kwargs or {})
    )
    parts = [r["out3"] for r in res.results]
    outv = gather_output(parts, b_proj)
    if _run_kwargs is not None:
        kernel.last_results = res
    return outv


# revision 67
# speedup vs baseline: 1.1863x; 1.0493x over previous
"""Trainium2 Bass kernel for nn_Attention (B=4, N=2048, C=768, H=12).

Sharding: 8 cores = 4 batches x 2 head-groups (6 heads each), Megatron-style
tensor parallel on the heads. Each core computes qkv for its head slice,
attention for 6 heads, and per-head-pair output-projection partials
out3 [3, 2048, 768] (bf16). The host sums the 3 pair partials of the 2 cores
covering each batch and adds the bias.

Design (optimized against the TimelineSim cost model, where a matmul costs
out_free_size cycles regardless of K, and the ACT engine costs free_size
cycles at 1.2GHz regardless of dtype):
  - ACT is the hard floor: 6 heads x 2048^2 softmax exps = 192 instructions
    of [128, 1024] ~= 199.5us busy. The whole schedule exists to keep the
    exp stream dense.
  - Everything bf16 on the PE (1 cyc/row, same as fp32r, half the DMA/SBUF).
  - x loaded ONCE (bf16, 4 strided DMAs) and resident; qkv passes re-read
    SBUF, not DRAM.
  - S^T tiles [128 j, 1024] = (1 j-chunk x 512 i x 2 heads); exp reads PSUM
    fp32, writes SBUF bf16.
  - PV is TOKEN-major: out[i, d] = P^T-chunk.T @ [V|1]: 65-row matmuls
    (vs 512-row feature-major) -> PV drops from 196k to 100k PE cycles.
    Z rides along as column 64 via a ones-column in v_sb.
  - normalize per token: reciprocal [128,4] + tensor_scalar_mul with
    per-partition 1/Z (no partition broadcast, no DRAM bounce), then a PE
    transpose (bf16, 128 cyc) flips [t, f] -> [f, t] for the projection.
  - The flat (pair, i4, j) S/exp stream is emitted directly; ALL other PE
    work (qkv passes, v production, PV, normalize, projections) flows
    through a budget-paced FIFO work queue drained between exps, so the
    in-order PE always runs S^T(idx+1) before PV(idx) and never starves ACT.
"""

import os
import sys
from contextlib import ExitStack

if "/opt/trn_rl_repo" not in sys.path:
    sys.path.insert(0, "/opt/trn_rl_repo")


def _env(name, default):
    return os.environ.get(name, default)

import numpy as np

import concourse.bass as bass
import concourse.mybir as mybir
import concourse.tile as tile
from concourse import bass_utils
from concourse.masks import make_identity

F32 = mybir.dt.float32
BF16 = mybir.dt.bfloat16

B, N, C = 4, 2048, 768
NH, D = 12, 64
SCALE = D ** -0.5
HPC = NH // 2          # heads per core
F = HPC * D            # 384 per-core features per projection
P = 128
CO = C // P            # 6 contraction chunks
FO = F // P            # 3 feature chunks (head pairs)
NO = N // P            # 16 token chunks of 128
NCORES = 8

# weight column sections (contiguous DMA reads): q0, k0, v, q12, k12
WSECS = ((0, P), (F, F + P), (2 * F, 3 * F), (P, F), (F + P, 2 * F))


def _split_multiwaits(nc):
    """This container's walrus accepts at most ONE sync-wait per instruction.

    Split any instruction carrying N>1 waits into (N-1) single-wait NOPs on
    the same engine queue placed immediately before it (engine queues are
    FIFO, so the semantics are identical)."""
    ctr = 0
    for f in nc.m.functions:
        for blk in f.blocks:
            insts = blk.instructions
            out = []
            changed = False
            for ins in insts:
                si = ins.sync_info
                if si is not None and len(si.on_wait) > 1:
                    changed = True
                    waits = list(si.on_wait)
                    for ww in waits[:-1]:
                        nop = mybir.InstNoOp(name=f"zzsplitw_{ctr}", ins=[], outs=[])
                        ctr += 1
                        nop.engine = ins.engine
                        nop.sync_info = mybir.SyncInfo(on_wait=[ww], on_update=[])
                        out.append(nop)
                    ins.sync_info = mybir.SyncInfo(
                        on_wait=[waits[-1]], on_update=list(si.on_update)
                    )
                out.append(ins)
            if changed:
                blk.instructions = out
    return nc


def _emit(nc, tc, ctx):
    # host pre-arranges everything partition-major so every input DMA is one
    # fully-contiguous >=1KB descriptor per partition (descriptors under 512B
    # pay a 2x latency multiplier in the DMA engines)
    xTb = nc.dram_tensor("xTb", [4, P, CO * 512], BF16, kind="ExternalInput").ap()
    wq_secs = {
        lo: nc.dram_tensor(f"wq{lo}", [P, CO * (hi - lo)], BF16,
                           kind="ExternalInput").ap()
        for lo, hi in WSECS
    }
    wprojT = nc.dram_tensor("wprojT", [P, FO * C], BF16, kind="ExternalInput").ap()
    out3 = nc.dram_tensor("out3", [FO, N, C], BF16, kind="ExternalOutput").ap()

    persist = ctx.enter_context(tc.tile_pool(name="persist", bufs=1))

    # x resident, one tile per 512-token slice: [128, co, 512]
    xt = [persist.tile([P, CO, 512], BF16, tag=f"x{n4}", name=f"x{n4}")
          for n4 in range(4)]
    # q/k in [feature, token] layout (heads packed in pairs per partition
    # group: head 2p -> partitions 0-63, head 2p+1 -> 64-127)
    q_sb = [[persist.tile([P, 512], BF16, tag=f"q{fo}_{n4}", name=f"q{fo}_{n4}")
             for n4 in range(4)] for fo in range(FO)]
    k_sb = [[persist.tile([P, 512], BF16, tag=f"k{fo}_{n4}", name=f"k{fo}_{n4}")
             for n4 in range(4)] for fo in range(FO)]
    # v in [token, head, feature+1] layout; column 64 is ones so the PV
    # matmul accumulates Z in out[:, 64]
    v_sb = [persist.tile([P, HPC, D + 1], BF16, tag=f"v{no}", name=f"v{no}")
            for no in range(NO)]
    # weights resident: one tile per section, [128, co, width]
    wq_sb = {lo: persist.tile([P, CO, hi - lo], BF16, tag=f"wq{lo}",
                              name=f"wq{lo}") for lo, hi in WSECS}
    wp_sb = persist.tile([P, FO, C], BF16, tag="wp")
    ident = persist.tile([P, P], BF16, tag="ident")
    # attention output per pair, [feature, token]; 2 rotating slots
    otp = ctx.enter_context(tc.tile_pool(name="otp", bufs=3))
    ot_sb = [otp.tile([P, N], BF16, tag="ot", name=f"ot{pr}") for pr in range(FO)]

    make_identity(nc, ident)
    for no in range(NO):
        nc.vector.memset(v_sb[no][:, :, D : D + 1], 1.0)
    # dummy exp: pulls the ACT table load into the DMA lead-in window
    expwarm = persist.tile([P, HPC], F32, tag="expwarm")
    nc.scalar.activation(
        out=expwarm,
        in_=ident[:, 0:HPC],
        func=mybir.ActivationFunctionType.Exp,
        scale=1.0,
    )

    with (
        tc.tile_pool(name="ptp", bufs=32) as pt_pool,
        tc.tile_pool(name="rzp", bufs=2) as rz_pool,
        tc.tile_pool(name="ntp", bufs=2) as nt_pool,
        tc.tile_pool(name="outp", bufs=3) as outp,
        tc.tile_pool(name="ps_st", bufs=2, space="PSUM") as ps_st,
        tc.tile_pool(name="ps_pv", bufs=1, space="PSUM") as ps_pv,
        tc.tile_pool(name="ps_mix", bufs=2, space="PSUM") as ps_mix,
    ):
        def wq_slice(foff, co, width=P):
            """AP for weight columns [foff, foff+width) of chunk co."""
            for lo, hi in WSECS:
                if lo <= foff and foff + width <= hi:
                    return wq_sb[lo][:, co, foff - lo : foff - lo + width]
            raise KeyError(foff)

        # NOTE: GPSIMD (Pool) cannot access PSUM on TRN2 (walrus BIR
        # verifier rejects it), so all psum->sbuf evacuations stay on DVE
        POOL_EVAC = _env("KERNEL_POOL_EVAC", "0") == "1"
        evac = nc.gpsimd if POOL_EVAC else nc.vector

        # ---- work-item thunks: (pe_ns, dve_ns, thunk) ------------------
        def qkv_group_thunks(fo, n4, which):
            """One q-or-k feature chunk for one 512-token slice, split into
            3 thunks of 2 matmuls (~430ns PE each)."""
            foff = fo * P if which == "q" else F + fo * P
            dst = (q_sb if which == "q" else k_sb)[fo][n4]
            state = {}

            def mk(c0):
                def th():
                    if c0 == 0:
                        state["pq"] = ps_mix.tile([P, 512], F32, tag="pqk",
                                                  name=f"pq_{which}{fo}_{n4}")
                    pq = state["pq"]
                    for co in (c0, c0 + 1):
                        nc.tensor.matmul(
                            pq,
                            wq_slice(foff, co),
                            xt[n4][:, co, :],
                            start=(co == 0),
                            stop=(co == CO - 1),
                        )
                    if c0 == CO - 2:
                        nc.vector.tensor_copy(out=dst, in_=pq)
                return th

            return [(427, 0, mk(0)), (427, 0, mk(2)), (427, 660, mk(4))]

        def v_chunk_thunks(no):
            """v for one 128-token chunk: 2 thunks of 3 accum matmuls."""
            state = {}

            def mk(c0):
                def th():
                    if c0 == 0:
                        state["pv"] = ps_mix.tile([P, F], F32, tag="pqk",
                                                  name=f"pv_{no}")
                    pv = state["pv"]
                    for co in (c0, c0 + 1, c0 + 2):
                        nc.tensor.matmul(
                            pv,
                            xt[no // 4][:, co, (no % 4) * P : (no % 4 + 1) * P],
                            wq_slice(2 * F, co, F),
                            start=(co == 0),
                            stop=(co == CO - 1),
                        )
                    if c0 == 3:
                        evac.tensor_copy(
                            out=v_sb[no][:, :, 0:D],
                            in_=pv.rearrange("p (h d) -> p h d", h=HPC),
                        )
                        v_done.add(no)
                return th

            return [(480, 0, mk(0)), (480, 0, mk(3))]

        def proj_chunk_thunks(pr, no):
            """Projection partial for one 128-token chunk of pair pr.
            Evacuation alternates DVE/Pool per chunk so consecutive chunks
            pipeline through two engines."""
            state = {}
            # final chunks run after the exp stream ends: alternate the
            # psum evacuations between DVE and the now-idle ACT engine
            if pr == 2 and no >= 12 and no % 2 == 1:
                ev = nc.scalar
            else:
                ev = nc.vector

            def mk(ob, width):
                def th():
                    if ob == 0:
                        state["o"] = outp.tile([P, C], BF16, tag="o",
                                               name=f"o_{pr}_{no}")
                    o_sb = state["o"]
                    pp = ps_mix.tile([P, 512], F32, tag="pqk",
                                     name=f"pp_{pr}_{no}_{ob}")
                    nc.tensor.matmul(
                        pp[:, 0:width],
                        ot_sb[pr][:, no * P : (no + 1) * P],
                        wp_sb[:, pr, ob * 512 : ob * 512 + width],
                        start=True,
                        stop=True,
                    )
                    ev.tensor_copy(
                        out=o_sb[:, ob * 512 : ob * 512 + width],
                        in_=pp[:, 0:width],
                    )
                    if ob == 1:
                        nc.sync.dma_start(
                            out=out3[pr, no * P : (no + 1) * P, :], in_=o_sb
                        )
                return th

            dve_c = 660 if no % 2 == 0 else 0
            return [(213, dve_c, mk(0, 512)), (107, dve_c // 2, mk(1, 256))]

        pt_map = {}
        v_done = set()

        # PSUM accumulation groups are exclusive per 2KB zero region (one
        # bank): only ONE group may be open in a bank at a time, and start=
        # True zeroes the whole region. So PV runs as FOUR sequential isub
        # passes per i4 block, each pass holding exactly one open group in
        # the pvA bank and one in the pvB bank; the per-isub normalize
        # (which reads the banks) runs between passes (WAR via the tile
        # pool rotation). The block's 16 exp tiles stay live in pt_pool
        # until its last pass.
        def pv_pass_thunks(pr, i4, isub, slot_pool=None):
            """One isub pass: accumulate 16 j-chunks into [128, 65] psum for
            each head, then normalize+transpose inline. The pass tile is
            [128, 130]: columns 0-64 hold the accumulation, 65-128 receive
            the PE transpose output (reusing the same bank keeps the pqk
            pool free of transpose traffic; the transpose's region-zeroing
            is safe because it reads nt, which the normalize muls produce
            AFTER they read the accumulation)."""
            hA, hB = 2 * pr, 2 * pr + 1
            st = {}

            def mk(t):
                def th():
                    if t == 0:
                        pool = slot_pool or ps_pv
                        tagA = "st" if slot_pool else "pvA"
                        tagB = "st" if slot_pool else "pvB"
                        st["A"] = pool.tile([P, 130], F32, tag=tagA,
                                            name=f"pvA_{pr}_{i4}_{isub}")
                        st["B"] = pool.tile([P, 130], F32, tag=tagB,
                                            name=f"pvB_{pr}_{i4}_{isub}")
                    pvA, pvB = st["A"], st["B"]
                    for j in range(4 * t, 4 * t + 4):
                        assert j in v_done, f"v({j}) not emitted before PV"
                        ptile = pt_map[(pr, i4, j)]
                        nc.tensor.matmul(
                            pvA[:, 0 : D + 1],
                            ptile[:, isub * P : (isub + 1) * P],
                            v_sb[j][:, hA, :],
                            start=(j == 0),
                            stop=(j == NO - 1),
                        )
                        nc.tensor.matmul(
                            pvB[:, 0 : D + 1],
                            ptile[:, 512 + isub * P : 512 + (isub + 1) * P],
                            v_sb[j][:, hB, :],
                            start=(j == 0),
                            stop=(j == NO - 1),
                        )
                    if t == 3:
                        st["nt"] = norm_muls(pr, i4, isub, pvA, pvB)
                return th

            def t_transpose():
                # separate queue item (~2 exp slots later) so the PE never
                # parks waiting for the normalize muls on DVE
                pvA = st["A"]
                nt = st["nt"]
                i0 = i4 * 512
                tpA = pvA[:, 66:130].bitcast(BF16)
                nc.tensor.transpose(tpA, nt, ident)
                nc.vector.tensor_copy(
                    out=ot_sb[pr][:, i0 + isub * P : i0 + (isub + 1) * P],
                    in_=tpA,
                )

            return [(220, 0, mk(0)), (220, 0, mk(1)), (220, 0, mk(2)),
                    (240, 1150, mk(3)), (60, 200, t_transpose)]

        def norm_muls(pr, i4, isub, pvA, pvB):
            rzA = rz_pool.tile([P, 1], F32, tag="rzA",
                               name=f"rzA_{pr}_{i4}_{isub}")
            rzB = rz_pool.tile([P, 1], F32, tag="rzB",
                               name=f"rzB_{pr}_{i4}_{isub}")
            nc.vector.reciprocal(out=rzA, in_=pvA[:, D : D + 1])
            nc.vector.reciprocal(out=rzB, in_=pvB[:, D : D + 1])
            nt = nt_pool.tile([P, P], BF16, tag="nt",
                              name=f"nt_{pr}_{i4}_{isub}")
            if pr == 2 and i4 == 3:
                # tail: the exp stream is over, ACT is idle — do the
                # normalize multiplies there (activation Copy with a
                # per-partition scale AP) to unload the DVE chain
                nc.scalar.mul(nt[:, 0:D], pvA[:, 0:D], rzA)
                nc.scalar.mul(nt[:, D:P], pvB[:, 0:D], rzB)
            else:
                nc.vector.tensor_scalar_mul(nt[:, 0:D], pvA[:, 0:D], rzA)
                nc.vector.tensor_scalar_mul(nt[:, D:P], pvB[:, 0:D], rzB)
            return nt

        # ---- DMA lead-in (few, large, contiguous DMAs; the three the first
        # matmuls need go out in parallel on separate engine queues) -------
        def dma_xt(n4, eng=None):
            (eng or nc.sync).dma_start(
                out=xt[n4].rearrange("p co c -> p (co c)"), in_=xTb[n4]
            )

        def dma_wq(lo, eng=None):
            (eng or nc.sync).dma_start(
                out=wq_sb[lo].rearrange("p co c -> p (co c)"), in_=wq_secs[lo]
            )

        dma_wq(0)                      # q0 weights (small) first
        dma_wq(F)                      # k0 weights
        for cp in range(3):            # x(n4=0) in co-pair pieces: the first
            nc.sync.dma_start(         # q00 matmuls start after ~1 piece
                out=xt[0][:, 2 * cp : 2 * cp + 2, :].rearrange(
                    "p co c -> p (co c)"),
                in_=xTb[0][:, cp * 1024 : (cp + 1) * 1024],
            )
        dma_wq(2 * F)                  # v weights
        dma_xt(1)
        dma_xt(2)
        dma_xt(3)
        dma_wq(P)                      # q1/q2
        dma_wq(F + P)                  # k1/k2
        nc.sync.dma_start(
            out=wp_sb.rearrange("p fo c -> p (fo c)"), in_=wprojT
        )

        # PE p-state warmup: dummy transposes keep the tensor engine busy
        # from t~0 so the clock is ramped when the real matmuls arrive
        for w in range(44):
            wtp = ps_st.tile([P, P], BF16, tag="st", name=f"warm{w}")
            nc.tensor.transpose(wtp, ident, ident)

        # minimal pre-attention compute: k0 quarter-0 first (so its psum
        # slot isn't stuck behind q00's evacuation in the pool rotation),
        # then q00, then the remaining k0 quarters. Each quarter gets its
        # OWN psum tile (one open accumulation group per 2KB zero region).
        def k00_quarter(quarter):
            sl = slice(quarter * P, (quarter + 1) * P)
            k00p = ps_mix.tile([P, P], F32, tag="pqk", name=f"k00p{quarter}")
            for co in range(CO):
                nc.tensor.matmul(
                    k00p,
                    wq_slice(F, co),
                    xt[0][:, co, sl],
                    start=(co == 0),
                    stop=(co == CO - 1),
                )
            nc.vector.tensor_copy(out=k_sb[0][0][:, sl], in_=k00p)

        k00_quarter(0)
        for _, _, th in qkv_group_thunks(0, 0, "q"):
            th()
        for quarter in range(1, 4):
            k00_quarter(quarter)

        # ---- EDF-ordered, budget-paced work pool -----------------------
        # Every non-S^T/exp instruction flows through one pool. Items carry
        # an execution DEADLINE in exp-index units (when their absence would
        # stall the exp stream: S^T operand production, pt-slot recycling
        # via PV, psum WAR via normalize) and an optional nbi (PV(j) may not
        # be emitted before exp(j)). drain() emits earliest-deadline-first,
        # rate-limited so the in-order PE always reaches the next S^T in
        # time; items with imminent deadlines are emitted regardless.
        import heapq

        pool = []          # heap of (deadline, seq, pe_cost, dve_cost, thunk)
        pending = {}       # nbi -> items
        seq_ctr = [0]

        def put(triples, deadline, nbi=None):
            for pe_c, dve_c, th in triples:
                item = (deadline, seq_ctr[0], pe_c, dve_c, th)
                seq_ctr[0] += 1
                if nbi is not None:
                    pending.setdefault(nbi, []).append(item)
                else:
                    heapq.heappush(pool, item)

        # virtual engine clocks: vpe/vdve track emitted PE/DVE-ns (S^T
        # included in vpe). Each is assumed never more than LAG behind the
        # ACT line (idx*1038) and floored to it; drains stop when either
        # line would push the next S^T past RESERVE before its exp slot.
        ACT_NS = 1038.0
        LAG = float(_env("KERNEL_LAG", 700))
        RESERVE = float(_env("KERNEL_RESERVE", 520))
        vpe = [0.0]
        vdve = [0.0]

        def drain(idx):
            for k in [k for k in pending if k <= idx]:
                for item in pending.pop(k):
                    heapq.heappush(pool, item)
            vpe[0] = max(vpe[0], idx * ACT_NS - LAG)
            vdve[0] = max(vdve[0], idx * ACT_NS - LAG)
            ceil = (idx + 1) * ACT_NS - RESERVE
            while pool:
                deadline, _, pe_c, dve_c, th = pool[0]
                if deadline > idx + 1 and (vpe[0] + pe_c > ceil
                                           or vdve[0] + dve_c > ceil):
                    break
                heapq.heappop(pool)
                th()
                vpe[0] += pe_c
                vdve[0] += dve_c

        def g_idx(pr, i4, j):
            return 64 * pr + 16 * i4 + j

        # ---- the flat S/exp stream with paced drains -------------------
        for pr in range(FO):
            for i4 in range(4):
                for j in range(NO):
                    gi = g_idx(pr, i4, j)
                    # work enqueues; deadlines are the exp idx by which the
                    # item must have EXECUTED to keep the exp stream dense
                    if pr == 0 and i4 == 0:
                        if j == 0:
                            put(qkv_group_thunks(0, 1, "k"), deadline=3)
                            put(qkv_group_thunks(0, 2, "k"), deadline=5)
                            put(qkv_group_thunks(0, 3, "k"), deadline=9)
                            put(qkv_group_thunks(0, 1, "q"), deadline=12)
                            put(qkv_group_thunks(0, 2, "q"), deadline=28)
                            put(qkv_group_thunks(0, 3, "q"), deadline=44)
                        # deadline strictly before any PV pass thunk (the
                        # passes read v_sb; emission order IS dependency
                        # order, so v must pop first) but after the early
                        # k/q groups the S^T stream needs
                        put(v_chunk_thunks(j), deadline=8 + 1.1 * j)
                    elif pr == 0 and i4 == 1 and j == 0:
                        for n4 in range(4):
                            put(qkv_group_thunks(1, n4, "k"),
                                deadline=62 + 4 * n4)
                        for n4 in range(4):
                            put(qkv_group_thunks(1, n4, "q"),
                                deadline=62 + 16 * n4)
                    elif pr == 1 and i4 == 1 and j == 0:
                        for n4 in range(4):
                            put(qkv_group_thunks(2, n4, "k"),
                                deadline=126 + 4 * n4)
                        for n4 in range(4):
                            put(qkv_group_thunks(2, n4, "q"),
                                deadline=126 + 16 * n4)
                    elif pr == 1 and i4 == 0 and j == 0:
                        for no in range(12):
                            put(proj_chunk_thunks(0, no), deadline=900 + no)
                    elif pr == 1 and i4 == 1 and j == 8:
                        # chunks 12-15 need ot[0] i4=3 (normalized early in
                        # pair 1) — enqueued later so EDF can't outrun it
                        for no in range(12, NO):
                            put(proj_chunk_thunks(0, no), deadline=900 + no)
                    elif pr == 2 and i4 == 0 and j == 0:
                        for no in range(12):
                            put(proj_chunk_thunks(1, no), deadline=920 + no)
                    elif pr == 2 and i4 == 1 and j == 8:
                        for no in range(12, NO):
                            put(proj_chunk_thunks(1, no), deadline=920 + no)
                    # the S^T + exp stream itself (emitted directly)
                    kt = k_sb[pr][j // 4]
                    jo = (j % 4) * P
                    qt = q_sb[pr][i4]
                    stm = ps_st.tile([P, 1024], F32, tag="st",
                                     name=f"st_{pr}_{i4}_{j}")
                    nc.tensor.matmul(
                        stm[:, 0:512], kt[0:64, jo : jo + P], qt[0:64, :],
                        start=True, stop=True,
                    )
                    nc.tensor.matmul(
                        stm[:, 512:1024], kt[64:128, jo : jo + P], qt[64:128, :],
                        start=True, stop=True,
                    )
                    ptile = pt_pool.tile([P, 1024], BF16, tag="pt",
                                         name=f"pt_{pr}_{i4}_{j}")
                    nc.scalar.activation(
                        out=ptile,
                        in_=stm,
                        func=mybir.ActivationFunctionType.Exp,
                        scale=SCALE,
                    )
                    pt_map[(pr, i4, j)] = ptile
                    vpe[0] += 427.0  # the two S^T matmuls above
                    drain(gi)
                # PV passes: strictly increasing deadlines keep the
                # pass/normalize WAR sequence ordered through the EDF heap.
                # On the final block the freed stm slots host passes 2/3 so
                # two pass chains run concurrently into the tail.
                g15 = g_idx(pr, i4, NO - 1)
                last_blk = (pr == 2 and i4 == 3)
                for isub in range(4):
                    sp = ps_st if (last_blk and isub >= 2) else None
                    ths = pv_pass_thunks(pr, i4, isub, slot_pool=sp)
                    if last_blk:
                        # two parallel chains {0,1} / {2,3}; first pass of
                        # each chain starts during the block's own exps
                        base = (g15 - 6 + 0.5 * (isub // 2) if isub % 2 == 0
                                else g15 + 2 + 0.5 * (isub // 2))
                    elif pr == 0:
                        # pair 0 is PE-oversubscribed early: push its PV
                        # into the next blocks (pt pool holds the tiles)
                        base = g15 + 12 + 4 * isub
                    else:
                        base = g15 + 2 + 4 * isub
                    dls = [base, base + 1, base + 2, base + 2.7, base + 3.7]
                    for t, (pe_c, dve_c, th) in enumerate(ths):
                        put([(pe_c, dve_c, th)], deadline=dls[t],
                            nbi=g_idx(pr, i4, min(4 * t + 3, NO - 1)) + 1)
                if pr == 2:
                    for c, no in enumerate(range(4 * i4, 4 * i4 + 4)):
                        put(proj_chunk_thunks(2, no),
                            deadline=g15 + 16 + 2.5 * c)
        # tail: everything left (final PVs, normalize, proj-2 chunks)
        for k in sorted(pending):
            for item in pending.pop(k):
                heapq.heappush(pool, item)
        while pool:
            heapq.heappop(pool)[4]()


_NC_CACHE = {}


def build_bass():
    key = "v3"
    if key in _NC_CACHE:
        return _NC_CACHE[key]
    nc = bass.Bass("TRN2")
    with tile.TileContext(nc) as tc:
        with ExitStack() as ctx:
            _emit(nc, tc, ctx)
    _split_multiwaits(nc)
    _NC_CACHE[key] = nc
    return nc


def make_in_maps(x, w_qkv, w_proj):
    import ml_dtypes

    bf16 = np.dtype(ml_dtypes.bfloat16)
    x = np.asarray(x, dtype=np.float32)
    w_qkv = np.asarray(w_qkv, dtype=np.float32)
    w_proj = np.asarray(w_proj, dtype=np.float32)
    wq, wk, wv = w_qkv[0:C], w_qkv[C : 2 * C], w_qkv[2 * C : 3 * C]
    in_maps = []
    for c in range(NCORES):
        b, g = divmod(c, 2)
        sl = slice(g * F, (g + 1) * F)
        wslice = np.concatenate([wq[sl], wk[sl], wv[sl]], axis=0)  # [1152, 768]
        wT = np.ascontiguousarray(wslice.T)  # [768, 1152]
        xT = x[b].T  # [768, 2048]
        # [n4, p, co*512]: per-partition rows fully contiguous
        xTb = np.ascontiguousarray(
            xT.reshape(CO, P, 4, 512).transpose(2, 1, 0, 3).reshape(4, P, CO * 512)
        ).astype(bf16)
        wpT = w_proj[:, sl].T  # [384, 768]
        m = {
            "xTb": xTb,
            "wprojT": np.ascontiguousarray(
                wpT.reshape(FO, P, C).transpose(1, 0, 2).reshape(P, FO * C)
            ).astype(bf16),
        }
        for lo, hi in WSECS:
            w = hi - lo
            m[f"wq{lo}"] = np.ascontiguousarray(
                wT[:, lo:hi].reshape(CO, P, w).transpose(1, 0, 2).reshape(P, CO * w)
            ).astype(bf16)
        in_maps.append(m)
    return in_maps


def gather_output(parts, b_proj):
    """parts: 8 arrays [FO, N, C] bf16 (pair partials per core)."""
    outv = np.empty((B, N, C), np.float32)
    for b in range(B):
        acc = parts[2 * b].astype(np.float32).sum(axis=0)
        acc += parts[2 * b + 1].astype(np.float32).sum(axis=0)
        outv[b] = acc
    outv += np.asarray(b_proj, dtype=np.float32)[None, None, :]
    return outv


def kernel(x, w_qkv, w_proj, b_proj, _run_kwargs=None):
    nc = build_bass()
    in_maps = make_in_maps(x, w_qkv, w_proj)
    res = bass_utils.run_bass_kernel_spmd(
        nc, in_maps, core_ids=list(range(NCORES)), **(_run_kwargs or {})
    )
    parts = [r["out3"] for r in res.results]
    outv = gather_output(parts, b_proj)
    if _run_kwargs is not None:
        kernel.last_results = res
    return outv
